# revision 1
# baseline (speedup 1.0000x reference)
"""GPT-1 forward (B=2,S=512,D=768,H=12,DFF=3072,L=12,V=32000) on 8 trn2 NeuronCores.

Strategy: sequence-parallel — 8 shards of 128 tokens (cores 0-3 = batch 0,
cores 4-7 = batch 1). Per layer each core computes Q/K/V for its tokens,
AllGathers K/V within its 4-core batch group, runs attention + FFN locally.
At the end, an 8-way AllGather of the residual stream feeds a vocab-sharded
output projection (each core computes a 4000-wide logit slice for all 1024
tokens). Weights are cast to bf16 on host; accumulation is fp32 in PSUM;
the residual stream is fp32 in SBUF.

Activations live feature-major [d, tok] in SBUF so every matmul uses the
natural [in, out] weight layout as the stationary operand, with no
transposes. Attention uses transposed scores [ktok, qtok]; softmax over the
partition axis is done with exp on ScalarE plus ones-matmul column sums and
a K=1 broadcast matmul on the TensorE (scores are small — no max-subtract
needed; 1/sqrt(dk) is folded into Wq on host).
"""

import numpy as np
import ml_dtypes

import concourse.bass as bass
import concourse.bacc as bacc
import concourse.tile as tile
import concourse.mybir as mybir
from concourse import bass_utils

dt = mybir.dt
F32 = dt.float32
BF16 = dt.bfloat16
NPBF = ml_dtypes.bfloat16
AF = mybir.ActivationFunctionType

B, S, D, H, DKH, DFF, L, V = 2, 512, 768, 12, 64, 3072, 12, 32000
NC = 8
TOK = (B * S) // NC          # 128 tokens per core
KT = D // 128                # 6 d-tiles
VSH = V // NC                # 4000 real vocab shard
VPAD = 4096                  # padded vocab shard
GROUPS = [[0, 1, 2, 3], [4, 5, 6, 7]]
ALL = [list(range(NC))]
LN_EPS = 1e-5

_cached = {}


def _build():
    if "nc" in _cached:
        return _cached["nc"]
    nc = bacc.Bacc(None, target_bir_lowering=False, num_devices=NC)

    x0_in = nc.dram_tensor("x0", [D, TOK], F32, kind="ExternalInput")
    wq_in = nc.dram_tensor("wq", [L, D, D], BF16, kind="ExternalInput")
    wk_in = nc.dram_tensor("wk", [L, D, D], BF16, kind="ExternalInput")
    wv_in = nc.dram_tensor("wv", [L, D, D], BF16, kind="ExternalInput")
    wo_in = nc.dram_tensor("wo", [L, D, D], BF16, kind="ExternalInput")
    w1_in = nc.dram_tensor("w1", [L, D, DFF], BF16, kind="ExternalInput")
    w2_in = nc.dram_tensor("w2", [L, DFF, D], BF16, kind="ExternalInput")
    wout_in = nc.dram_tensor("wout", [D, VPAD], BF16, kind="ExternalInput")
    out_d = nc.dram_tensor("logits", [VPAD, B * S], F32, kind="ExternalOutput")

    with tile.TileContext(nc) as tc:
        with (
            tc.tile_pool(name="res", bufs=1) as res,
            tc.tile_pool(name="psA", bufs=6, space="PSUM") as psA,
            tc.tile_pool(name="psL", bufs=2, space="PSUM") as psL,
            tc.tile_pool(name="dram", bufs=2, space="DRAM") as dram,
        ):
            # constants
            ones_col_f = res.tile([128, 1], F32)
            nc.gpsimd.memset(ones_col_f[:], 1.0)
            ones_col_b = res.tile([128, 1], BF16)
            nc.gpsimd.memset(ones_col_b[:], 1.0)
            ones_row_f = res.tile([1, 128], F32)
            nc.gpsimd.memset(ones_row_f[:], 1.0)
            eps_sb = res.tile([1, 1], F32)
            nc.gpsimd.memset(eps_sb[:], LN_EPS)

            # residual stream, feature-major [128, kt, tok] fp32
            x_sb = res.tile([128, KT, TOK], F32)
            nc.sync.dma_start(x_sb[:], x0_in[:].rearrange("(t p) n -> p t n", p=128))

            def layernorm(act, x2_out):
                """x2_out (bf16) = normalize(x_sb) ; no scale/bias (always 1/0)."""
                mu_ps = psA.tile([1, TOK], F32, tag="mm")
                for kt in range(KT):
                    nc.tensor.matmul(mu_ps[:], ones_col_f[:], x_sb[:, kt, :],
                                     start=kt == 0, stop=kt == KT - 1)
                sq = act.tile([128, KT, TOK], F32, tag="sq")
                for kt in range(KT):
                    nc.vector.tensor_mul(sq[:, kt, :], x_sb[:, kt, :], x_sb[:, kt, :])
                s2_ps = psA.tile([1, TOK], F32, tag="mm")
                for kt in range(KT):
                    nc.tensor.matmul(s2_ps[:], ones_col_f[:], sq[:, kt, :],
                                     start=kt == 0, stop=kt == KT - 1)
                mu = act.tile([1, TOK], F32, tag="mu")
                nc.vector.tensor_scalar_mul(mu[:], mu_ps[:], 1.0 / D)
                msq = act.tile([1, TOK], F32, tag="msq")
                nc.vector.tensor_scalar_mul(msq[:], s2_ps[:], 1.0 / D)
                mu2 = act.tile([1, TOK], F32, tag="mu2")
                nc.vector.tensor_mul(mu2[:], mu[:], mu[:])
                var = act.tile([1, TOK], F32, tag="var")
                nc.vector.tensor_sub(var[:], msq[:], mu2[:])
                sd = act.tile([1, TOK], F32, tag="sd")
                nc.scalar.activation(sd[:], var[:], AF.Sqrt, bias=eps_sb[:], scale=1.0)
                rstd = act.tile([1, TOK], F32, tag="rstd")
                nc.vector.reciprocal(rstd[:], sd[:])
                bmu = psA.tile([128, TOK], F32, tag="mm")
                nc.tensor.matmul(bmu[:], ones_row_f[:], mu[:], start=True, stop=True)
                brs = psA.tile([128, TOK], F32, tag="mm")
                nc.tensor.matmul(brs[:], ones_row_f[:], rstd[:], start=True, stop=True)
                tmp = act.tile([128, KT, TOK], F32, tag="lntmp")
                for kt in range(KT):
                    nc.vector.tensor_sub(tmp[:, kt, :], x_sb[:, kt, :], bmu[:])
                for kt in range(KT):
                    nc.vector.tensor_mul(x2_out[:, kt, :], tmp[:, kt, :], brs[:])

            def wproj_into(w_sb, rhs_sb, n_f, sink, nk=KT):
                """out[f,tok] += W.T @ rhs ; sink(ft, psum_tile)."""
                for ft in range(n_f):
                    ps = psA.tile([128, TOK], F32, tag="mm")
                    for kt in range(nk):
                        nc.tensor.matmul(ps[:], w_sb[:, kt, ft * 128:(ft + 1) * 128],
                                         rhs_sb[:, kt, :], start=kt == 0, stop=kt == nk - 1)
                    sink(ft, ps)

            with (
                tc.tile_pool(name="wqkvo", bufs=2) as wp,
                tc.tile_pool(name="w1p", bufs=2) as w1p,
                tc.tile_pool(name="w2p", bufs=2) as w2p,
                tc.tile_pool(name="act", bufs=1) as act,
                tc.tile_pool(name="hd", bufs=3) as hd,
            ):
                for l in range(L):
                    wq_sb = wp.tile([128, KT, D], BF16, tag="wq")
                    wk_sb = wp.tile([128, KT, D], BF16, tag="wk")
                    wv_sb = wp.tile([128, KT, D], BF16, tag="wv")
                    wo_sb = wp.tile([128, KT, D], BF16, tag="wo")
                    for w_sb, w_in in ((wk_sb, wk_in), (wv_sb, wv_in), (wq_sb, wq_in), (wo_sb, wo_in)):
                        nc.sync.dma_start(w_sb[:], w_in[l].rearrange("(t p) f -> p t f", p=128))

                    # ---- LN1 ----
                    x2_sb = act.tile([128, KT, TOK], BF16, tag="x2")
                    layernorm(act, x2_sb)

                    # ---- allgather x2; K,V computed for all 512 tokens locally ----
                    x2b = dram.tile([D, TOK], BF16, tag="x2b")
                    nc.sync.dma_start(x2b[:].rearrange("(t p) n -> p t n", p=128), x2_sb[:])
                    x2g = dram.tile([4 * D, TOK], BF16, tag="x2g")
                    nc.gpsimd.collective_compute(
                        "AllGather", mybir.AluOpType.bypass, replica_groups=GROUPS,
                        ins=[x2b.opt()], outs=[x2g.opt()])

                    # Q projection from local x2 overlaps the AllGather
                    q_sb = act.tile([128, KT, TOK], BF16, tag="q")
                    wproj_into(wq_sb, x2_sb, KT,
                               lambda ft, ps: nc.vector.tensor_copy(q_sb[:, ft, :], ps[:]))

                    x2f = act.tile([128, KT, 4 * TOK], BF16, tag="x2f")
                    x2g_r = x2g[:].rearrange("(r t p) n -> r p t n", r=4, t=KT)
                    for r in range(4):
                        nc.sync.dma_start(x2f[:, :, r * TOK:(r + 1) * TOK], x2g_r[r])
                    k_sb = act.tile([128, KT, 4 * TOK], BF16, tag="k")
                    for ft in range(KT):
                        ps = psL.tile([128, 512], F32, tag="lg")
                        for kt in range(KT):
                            nc.tensor.matmul(ps[:], wk_sb[:, kt, ft * 128:(ft + 1) * 128],
                                             x2f[:, kt, :], start=kt == 0, stop=kt == KT - 1)
                        nc.vector.tensor_copy(k_sb[:, ft, :], ps[:])
                    v_sb = act.tile([128, 4, D], BF16, tag="v")  # [tok128, rank, d]
                    for c in range(4):
                        for fc, fw in ((0, 512), (512, 256)):
                            ps = psL.tile([128, fw], F32, tag="lg")
                            for kt in range(KT):
                                nc.tensor.matmul(ps[:], x2f[:, kt, c * TOK:(c + 1) * TOK],
                                                 wv_sb[:, kt, fc:fc + fw],
                                                 start=kt == 0, stop=kt == KT - 1)
                            nc.vector.tensor_copy(v_sb[:, c, fc:fc + fw], ps[:])

                    # ---- attention (k/v resident in SBUF) ----
                    ctx_sb = act.tile([128, KT, TOK], BF16, tag="ctx")
                    for j in range(KT):       # head pair j -> ctx d-tile j
                        ctx_ps = psA.tile([128, TOK], F32, tag="mm")
                        for hh in range(2):
                            h, ro = 2 * j + hh, hh * 64
                            e_sb = hd.tile([128, 4, TOK], BF16, tag="e")
                            den = psA.tile([1, TOK], F32, tag="mm")
                            for kt in range(4):
                                st = psA.tile([128, TOK], F32, tag="mm")
                                nc.tensor.matmul(st[:], k_sb[ro:ro + 64, j, kt * 128:(kt + 1) * 128],
                                                 q_sb[ro:ro + 64, j, :], start=True, stop=True)
                                nc.scalar.activation(e_sb[:, kt, :], st[:], AF.Exp)
                            for kt in range(4):
                                nc.tensor.matmul(den[:], ones_col_b[:], e_sb[:, kt, :],
                                                 start=kt == 0, stop=kt == 3)
                            rec = hd.tile([1, TOK], F32, tag="rec")
                            nc.vector.reciprocal(rec[:], den[:])
                            bre = psA.tile([128, TOK], F32, tag="mm")
                            nc.tensor.matmul(bre[:], ones_row_f[:], rec[:],
                                             start=True, stop=True)
                            bre_sb = hd.tile([128, TOK], F32, tag="bres")
                            nc.vector.tensor_copy(bre_sb[:], bre[:])
                            for kt in range(4):
                                nc.tensor.matmul(ctx_ps[ro:ro + 64, :],
                                                 v_sb[:, kt, h * 64:(h + 1) * 64],
                                                 e_sb[:, kt, :], start=kt == 0, stop=kt == 3,
                                                 tile_position=(0, ro))
                            nc.vector.tensor_mul(ctx_sb[ro:ro + 64, j, :],
                                                 bre_sb[ro:ro + 64, :], ctx_ps[ro:ro + 64, :])

                    # ---- Wo + residual ----
                    def wo_sink(ft, ps):
                        nc.vector.tensor_add(x_sb[:, ft, :], x_sb[:, ft, :], ps[:])
                    wproj_into(wo_sb, ctx_sb, KT, wo_sink)

                    # ---- LN2 + FFN ----
                    x2_sb = act.tile([128, KT, TOK], BF16, tag="x2")
                    layernorm(act, x2_sb)
                    h_sb = act.tile([128, DFF // 128, TOK], BF16, tag="h")
                    for c in range(2):
                        w1c = w1p.tile([128, KT, DFF // 2], BF16, tag="w1")
                        nc.sync.dma_start(
                            w1c[:], w1_in[l][:, c * (DFF // 2):(c + 1) * (DFF // 2)]
                            .rearrange("(t p) f -> p t f", p=128))
                        def g_sink(ft, ps, c=c):
                            nc.scalar.activation(h_sb[:, c * 12 + ft, :], ps[:], AF.Gelu)
                        wproj_into(w1c, x2_sb, 12, g_sink)
                    w2c0 = w2p.tile([128, 12, D], BF16, tag="w2")
                    w2c1 = w2p.tile([128, 12, D], BF16, tag="w2")
                    for c, w2c in enumerate((w2c0, w2c1)):
                        nc.sync.dma_start(
                            w2c[:], w2_in[l][c * (DFF // 2):(c + 1) * (DFF // 2), :]
                            .rearrange("(t p) f -> p t f", p=128))
                    for ft in range(KT):
                        ps = psA.tile([128, TOK], F32, tag="mm")
                        for kt in range(DFF // 128):
                            w2c = (w2c0, w2c1)[kt // 12]
                            nc.tensor.matmul(ps[:], w2c[:, kt % 12, ft * 128:(ft + 1) * 128],
                                             h_sb[:, kt, :], start=kt == 0, stop=kt == DFF // 128 - 1)
                        nc.vector.tensor_add(x_sb[:, ft, :], x_sb[:, ft, :], ps[:])

            # ---- final: allgather x, vocab-sharded projection ----
            with (
                tc.tile_pool(name="fin", bufs=1) as fin,
                tc.tile_pool(name="wop", bufs=2) as wop,
                tc.tile_pool(name="lgp", bufs=3) as lgp,
            ):
                xb_sb = fin.tile([128, KT, TOK], BF16)
                for kt in range(KT):
                    nc.vector.tensor_copy(xb_sb[:, kt, :], x_sb[:, kt, :])
                xf = dram.tile([D, TOK], BF16, tag="xf")
                nc.sync.dma_start(xf[:].rearrange("(t p) n -> p t n", p=128), xb_sb[:])
                xg = dram.tile([NC * D, TOK], BF16, tag="xg", addr_space="Shared")
                nc.gpsimd.collective_compute(
                    "AllGather", mybir.AluOpType.bypass, replica_groups=ALL,
                    ins=[xf.opt()], outs=[xg.opt()])
                xg_sb = fin.tile([128, KT, NC, TOK], BF16)
                xg_r = xg[:].rearrange("(r t p) n -> r p t n", r=NC, t=KT)
                for r in range(NC):
                    nc.sync.dma_start(xg_sb[:, :, r, :], xg_r[r])
                for c in range(4):
                    woc = wop.tile([128, KT, 1024], BF16, tag="wout")
                    nc.sync.dma_start(
                        woc[:], wout_in[:, c * 1024:(c + 1) * 1024]
                        .rearrange("(t p) f -> p t f", p=128))
                    for vt in range(8):
                        for hf in range(2):
                            ps = psL.tile([128, 512], F32, tag="lg")
                            for kt in range(KT):
                                nc.tensor.matmul(
                                    ps[:], woc[:, kt, vt * 128:(vt + 1) * 128],
                                    xg_sb[:, kt, hf * 4:(hf + 1) * 4, :],
                                    start=kt == 0, stop=kt == KT - 1)
                            lg = lgp.tile([128, 512], F32, tag="lgo")
                            nc.scalar.copy(lg[:], ps[:])
                            nc.sync.dma_start(
                                out_d[(c * 8 + vt) * 128:(c * 8 + vt + 1) * 128,
                                      hf * 512:(hf + 1) * 512], lg[:])
    nc.compile()
    _cached["nc"] = nc
    return nc


def _prep_inputs(inputs):
    tok = np.asarray(inputs["tokens"])
    x0 = np.asarray(inputs["tok_emb"], np.float32)[tok] + np.asarray(inputs["pos_emb"], np.float32)[None]
    x0 = x0.reshape(B * S, D)

    for name in ("bq", "bk", "bv", "bo", "b1", "b2", "b_out", "ln1_b", "ln2_b"):
        assert not np.any(np.asarray(inputs[name])), f"{name} expected to be all zeros"
    for name in ("ln1_s", "ln2_s"):
        assert np.all(np.asarray(inputs[name]) == 1.0), f"{name} expected to be all ones"

    cast = lambda a: np.ascontiguousarray(np.asarray(a, np.float32)).astype(NPBF)
    wq = cast(np.asarray(inputs["Wq"], np.float32) / np.sqrt(DKH))
    wk = cast(inputs["Wk"])
    wv = cast(inputs["Wv"])
    wo = cast(inputs["Wo"])
    w1 = cast(inputs["W1"])
    w2 = cast(inputs["W2"])
    wout_full = np.zeros((D, VPAD * NC // NC * NC), np.float32)  # [D, 32768] padded
    wout_full = np.zeros((D, NC * VPAD), np.float32)
    wout_full[:, :0] = 0  # noop
    wout = np.asarray(inputs["W_out"], np.float32)

    in_maps = []
    for c in range(NC):
        wc = np.zeros((D, VPAD), np.float32)
        wc[:, :VSH] = wout[:, c * VSH:(c + 1) * VSH]
        in_maps.append({
            "x0": np.ascontiguousarray(x0[c * TOK:(c + 1) * TOK].T),
            "wq": wq, "wk": wk, "wv": wv, "wo": wo, "w1": w1, "w2": w2,
            "wout": wc.astype(NPBF),
        })
    return in_maps


def _assemble(results):
    parts = [np.asarray(results[c]["logits"][:VSH]) for c in range(NC)]
    logits = np.concatenate(parts, axis=0)          # [V, B*S]
    return np.ascontiguousarray(logits.T).reshape(B, S, V).astype(np.float32)


def _run(inputs, **kw):
    nc = _build()
    in_maps = _prep_inputs(inputs)
    res = bass_utils.run_bass_kernel_spmd(nc, in_maps, core_ids=list(range(NC)), **kw)
    return _assemble(res.results), res


def kernel(**inputs):
    out, _ = _run(inputs)
    return out



# revision 4
# speedup vs baseline: 1.0163x; 1.0163x over previous
"""GPT-1 forward (B=2,S=512,D=768,H=12,DFF=3072,L=12,V=32000) on 8 trn2 NeuronCores.

Strategy: sequence-parallel — 8 shards of 128 tokens (cores 0-3 = batch 0,
cores 4-7 = batch 1). Per layer each core computes Q/K/V for its tokens,
AllGathers the LN1 output within its 4-core batch group, recomputes K/V for
all 512 group tokens locally, runs attention + FFN on its 128 tokens. At the
end an 8-way AllGather of the residual feeds a vocab-sharded output
projection (each core computes a 4000-wide logit slice for all 1024 tokens).

v2 performance structure:
- All weights are packed on host into SBUF layout [128, flat] so every
  weight load is one fully-contiguous-per-partition DMA (3 per layer).
  Weight pools are single-buffered; next-layer DMAs are issued right after
  the current layer's last read so transfers overlap compute.
- Activations live feature-major [d, tok]; matmuls use natural [in, out]
  weights as stationary operand.
- Attention scores / FFN W1 accumulate into [128,512] PSUM banks (4 chains
  side by side) so exp/gelu run as one [128,512] activation each.
- LayerNorm stats run on VectorE (sqrt/reciprocal) — ScalarE only ever
  loads the exp and gelu tables (2 switches/layer instead of 4+).
- Final logits are staged in SBUF and stored with one 4MB DMA per quarter.
"""

import numpy as np
import ml_dtypes

import concourse.bass as bass
import concourse.bacc as bacc
import concourse.tile as tile
import concourse.mybir as mybir
from concourse import bass_utils

dt = mybir.dt
F32 = dt.float32
BF16 = dt.bfloat16
NPBF = ml_dtypes.bfloat16
AF = mybir.ActivationFunctionType

B, S, D, H, DKH, DFF, L, V = 2, 512, 768, 12, 64, 3072, 12, 32000
NC = 8
TOK = (B * S) // NC          # 128 tokens per core
KT = D // 128                # 6 d-tiles
KT2 = DFF // 128             # 24 dff-tiles
VSH = V // NC                # 4000 real vocab shard
VPAD = 4096                  # padded vocab shard
GROUPS = [[0, 1, 2, 3], [4, 5, 6, 7]]
ALL = [list(range(NC))]
LN_EPS = 1e-5

_cached = {}


def _build():
    if "nc" in _cached:
        return _cached["nc"]
    nc = bacc.Bacc(None, target_bir_lowering=False, num_devices=NC)

    x0_in = nc.dram_tensor("x0", [128, KT * TOK], F32, kind="ExternalInput")
    wa_in = nc.dram_tensor("wa", [L, 128, KT * 4 * D], BF16, kind="ExternalInput")
    w1_in = nc.dram_tensor("w1", [L, 128, KT * DFF], BF16, kind="ExternalInput")
    w2_in = nc.dram_tensor("w2", [L, 128, KT2 * D], BF16, kind="ExternalInput")
    wout_in = nc.dram_tensor("wout", [4, 128, KT * 1024], BF16, kind="ExternalInput")
    out_d = nc.dram_tensor("logits", [VPAD, B * S], F32, kind="ExternalOutput")

    with tile.TileContext(nc) as tc:
        with (
            tc.tile_pool(name="res", bufs=1) as res,
            tc.tile_pool(name="psA", bufs=3, space="PSUM") as psA,
            tc.tile_pool(name="psB", bufs=2, space="PSUM") as psB,
            tc.tile_pool(name="psW", bufs=3, space="PSUM") as psW,
            tc.tile_pool(name="dram", bufs=2, space="DRAM") as dram,
        ):
            # constants
            ones_col_f = res.tile([128, 1], F32)
            nc.gpsimd.memset(ones_col_f[:], 1.0)
            ones_col_b = res.tile([128, 1], BF16)
            nc.gpsimd.memset(ones_col_b[:], 1.0)
            ones_row_f = res.tile([1, 128], F32)
            nc.gpsimd.memset(ones_row_f[:], 1.0)

            # residual stream, feature-major [128, kt, tok] fp32
            x_sb = res.tile([128, KT, TOK], F32)
            nc.sync.dma_start(x_sb[:], x0_in[:].rearrange("p (t n) -> p t n", t=KT))

            def layernorm(act, x2_out):
                """x2_out (bf16) = normalize(x_sb); no scale/bias (ones/zeros).
                Stats via ones-matmuls; rstd via bit-hack rsqrt + 2 Newton
                steps, all on VectorE (no ScalarE table loads)."""
                mu_ps = psA.tile([1, TOK], F32, tag="mm")
                for kt in range(KT):
                    nc.tensor.matmul(mu_ps[:], ones_col_f[:], x_sb[:, kt, :],
                                     start=kt == 0, stop=kt == KT - 1)
                sq = act.tile([128, KT, TOK], F32, tag="sq")
                for kt in range(KT):
                    nc.vector.tensor_mul(sq[:, kt, :], x_sb[:, kt, :], x_sb[:, kt, :])
                s2_ps = psA.tile([1, TOK], F32, tag="mm")
                for kt in range(KT):
                    nc.tensor.matmul(s2_ps[:], ones_col_f[:], sq[:, kt, :],
                                     start=kt == 0, stop=kt == KT - 1)
                mu = act.tile([1, TOK], F32, tag="mu")
                nc.vector.tensor_scalar_mul(mu[:], mu_ps[:], 1.0 / D)
                mu2 = act.tile([1, TOK], F32, tag="mu2")
                nc.vector.tensor_mul(mu2[:], mu[:], mu[:])
                var = act.tile([1, TOK], F32, tag="var")
                # var + eps = s2/D - mu^2 + eps
                nc.vector.tensor_scalar(var[:], s2_ps[:], 1.0 / D, LN_EPS,
                                        mybir.AluOpType.mult, mybir.AluOpType.add)
                nc.vector.tensor_sub(var[:], var[:], mu2[:])
                ti = act.tile([1, TOK], dt.int32, tag="ti")
                nc.vector.tensor_scalar(ti[:], var[:].bitcast(dt.int32), 1, None,
                                        mybir.AluOpType.logical_shift_right)
                nc.vector.tensor_scalar(ti[:], ti[:], -1, 0x5F3759DF,
                                        mybir.AluOpType.mult, mybir.AluOpType.add)
                y0 = ti[:].bitcast(F32)
                nt = act.tile([1, TOK], F32, tag="nt")
                rstd = act.tile([1, TOK], F32, tag="rstd")
                nc.vector.tensor_mul(nt[:], y0, y0)
                nc.vector.tensor_mul(nt[:], nt[:], var[:])
                nc.vector.tensor_scalar(nt[:], nt[:], -0.5, 1.5,
                                        mybir.AluOpType.mult, mybir.AluOpType.add)
                nc.vector.tensor_mul(rstd[:], nt[:], y0)
                nc.vector.tensor_mul(nt[:], rstd[:], rstd[:])
                nc.vector.tensor_mul(nt[:], nt[:], var[:])
                nc.vector.tensor_scalar(nt[:], nt[:], -0.5, 1.5,
                                        mybir.AluOpType.mult, mybir.AluOpType.add)
                nc.vector.tensor_mul(rstd[:], nt[:], rstd[:])
                bmu = psB.tile([128, TOK], F32, tag="bc")
                nc.tensor.matmul(bmu[:], ones_row_f[:], mu[:], start=True, stop=True)
                brs = psB.tile([128, TOK], F32, tag="bc")
                nc.tensor.matmul(brs[:], ones_row_f[:], rstd[:], start=True, stop=True)
                tmp = act.tile([128, KT, TOK], F32, tag="lntmp")
                for kt in range(KT):
                    nc.vector.tensor_sub(tmp[:, kt, :], x_sb[:, kt, :], bmu[:])
                for kt in range(KT):
                    nc.vector.tensor_mul(x2_out[:, kt, :], tmp[:, kt, :], brs[:])

            with (
                tc.tile_pool(name="wap", bufs=1) as wap,
                tc.tile_pool(name="w1p", bufs=1) as w1p,
                tc.tile_pool(name="w2p", bufs=1) as w2p,
                tc.tile_pool(name="act", bufs=1) as act,
                tc.tile_pool(name="hd", bufs=3) as hd,
            ):
                def load_wa(l):
                    w = wap.tile([128, KT, 4, D], BF16, tag="wa")
                    nc.sync.dma_start(
                        w[:], wa_in[l].rearrange("p (t i f) -> p t i f", t=KT, i=4))
                    return w

                def load_w1(l):
                    w = w1p.tile([128, KT, DFF], BF16, tag="w1")
                    nc.sync.dma_start(
                        w[:], w1_in[l].rearrange("p (t f) -> p t f", t=KT))
                    return w

                def load_w2(l):
                    w = w2p.tile([128, KT2, D], BF16, tag="w2")
                    nc.sync.dma_start(
                        w[:], w2_in[l].rearrange("p (t f) -> p t f", t=KT2))
                    return w

                wa_sb = load_wa(0)
                w1_sb = load_w1(0)
                w2_sb = load_w2(0)

                for l in range(L):
                    # ---- LN1 ----
                    x2_sb = act.tile([128, KT, TOK], BF16, tag="x2")
                    layernorm(act, x2_sb)

                    # ---- allgather x2 within 4-core batch group ----
                    x2b = dram.tile([128, KT * TOK], BF16, tag="x2b")
                    nc.sync.dma_start(x2b[:], x2_sb[:].rearrange("p t n -> p (t n)"))
                    x2g = dram.tile([4 * 128, KT * TOK], BF16, tag="x2g")
                    nc.gpsimd.collective_compute(
                        "AllGather", mybir.AluOpType.bypass, replica_groups=GROUPS,
                        ins=[x2b.opt()], outs=[x2g.opt()])

                    # Q projection from local x2 overlaps the AllGather
                    q_sb = act.tile([128, KT, TOK], BF16, tag="q")
                    for ft in range(KT):
                        ps = psA.tile([128, TOK], F32, tag="mm")
                        for kt in range(KT):
                            nc.tensor.matmul(ps[:], wa_sb[:, kt, 0, ft * 128:(ft + 1) * 128],
                                             x2_sb[:, kt, :], start=kt == 0, stop=kt == KT - 1)
                        nc.vector.tensor_copy(q_sb[:, ft, :], ps[:])

                    # gathered x2 for the whole group: [p, rank, t, n]
                    x2f = act.tile([128, 4, KT, TOK], BF16, tag="x2f")
                    nc.sync.dma_start(x2f[:], x2g[:].rearrange(
                        "(r p) (t n) -> p r t n", p=128, t=KT))

                    # K for all 512 group tokens: [128, ft, 4*TOK]
                    k_sb = act.tile([128, KT, 4 * TOK], BF16, tag="k")
                    for ft in range(KT):
                        ps = psW.tile([128, 512], F32, tag="lg")
                        for kt in range(KT):
                            nc.tensor.matmul(ps[:], wa_sb[:, kt, 1, ft * 128:(ft + 1) * 128],
                                             x2f[:, :, kt, :], start=kt == 0, stop=kt == KT - 1)
                        nc.vector.tensor_copy(k_sb[:, ft, :], ps[:])

                    # V token-major [tok128, rank, d]
                    v_sb = act.tile([128, 4, D], BF16, tag="v")
                    for c in range(4):
                        for fc, fw in ((0, 512), (512, 256)):
                            ps = psW.tile([128, fw], F32, tag="lg")
                            for kt in range(KT):
                                nc.tensor.matmul(ps[:], x2f[:, c, kt, :],
                                                 wa_sb[:, kt, 2, fc:fc + fw],
                                                 start=kt == 0, stop=kt == KT - 1)
                            nc.vector.tensor_copy(v_sb[:, c, fc:fc + fw], ps[:])

                    # ---- attention ----
                    ctx_sb = act.tile([128, KT, TOK], BF16, tag="ctx")
                    for j in range(KT):       # head pair j -> ctx d-tile j
                        ctx_ps = psA.tile([128, TOK], F32, tag="mm")
                        for hh in range(2):
                            h, ro = 2 * j + hh, hh * 64
                            st = psW.tile([128, 4 * TOK], F32, tag="lg")
                            for kt in range(4):
                                nc.tensor.matmul(st[:, kt * TOK:(kt + 1) * TOK],
                                                 k_sb[ro:ro + 64, j, kt * 128:(kt + 1) * 128],
                                                 q_sb[ro:ro + 64, j, :], start=True, stop=True)
                            e_sb = hd.tile([128, 4 * TOK], BF16, tag="e")
                            nc.scalar.activation(e_sb[:], st[:], AF.Exp)
                            den = psA.tile([1, TOK], F32, tag="mm")
                            for kt in range(4):
                                nc.tensor.matmul(den[:], ones_col_b[:], e_sb[:, kt * TOK:(kt + 1) * TOK],
                                                 start=kt == 0, stop=kt == 3)
                            rec = hd.tile([1, TOK], F32, tag="rec")
                            nc.vector.reciprocal(rec[:], den[:])
                            bre = psB.tile([128, TOK], F32, tag="bc")
                            nc.tensor.matmul(bre[:], ones_row_f[:], rec[:],
                                             start=True, stop=True)
                            bre_sb = hd.tile([128, TOK], F32, tag="bres")
                            nc.scalar.copy(bre_sb[:], bre[:])
                            for kt in range(4):
                                nc.tensor.matmul(ctx_ps[ro:ro + 64, :],
                                                 v_sb[:, kt, h * 64:(h + 1) * 64],
                                                 e_sb[:, kt * TOK:(kt + 1) * TOK],
                                                 start=kt == 0, stop=kt == 3,
                                                 tile_position=(0, ro))
                            nc.vector.tensor_mul(ctx_sb[ro:ro + 64, j, :],
                                                 bre_sb[ro:ro + 64, :], ctx_ps[ro:ro + 64, :])

                    # ---- Wo + residual ----
                    for ft in range(KT):
                        ps = psA.tile([128, TOK], F32, tag="mm")
                        for kt in range(KT):
                            nc.tensor.matmul(ps[:], wa_sb[:, kt, 3, ft * 128:(ft + 1) * 128],
                                             ctx_sb[:, kt, :], start=kt == 0, stop=kt == KT - 1)
                        nc.vector.tensor_add(x_sb[:, ft, :], x_sb[:, ft, :], ps[:])

                    # prefetch next layer's attention weights (wa slot now free)
                    if l + 1 < L:
                        wa_next = load_wa(l + 1)

                    # ---- LN2 + FFN ----
                    x2_sb = act.tile([128, KT, TOK], BF16, tag="x2")
                    layernorm(act, x2_sb)

                    # W1 + gelu: 6 chunks of 4 sequential chains in one bank
                    h_sb = act.tile([128, KT2 * TOK], BF16, tag="h")
                    for ch in range(6):
                        ps = psW.tile([128, 512], F32, tag="lg")
                        for sub in range(4):
                            ft = ch * 4 + sub
                            for kt in range(KT):
                                nc.tensor.matmul(
                                    ps[:, sub * 128:(sub + 1) * 128],
                                    w1_sb[:, kt, ft * 128:(ft + 1) * 128],
                                    x2_sb[:, kt, :], start=kt == 0, stop=kt == KT - 1)
                        nc.scalar.activation(
                            h_sb[:, ch * 512:(ch + 1) * 512], ps[:], AF.Gelu)

                    if l + 1 < L:
                        w1_next = load_w1(l + 1)

                    # W2 + residual
                    for ft in range(KT):
                        ps = psA.tile([128, TOK], F32, tag="mm")
                        for kt in range(KT2):
                            nc.tensor.matmul(ps[:], w2_sb[:, kt, ft * 128:(ft + 1) * 128],
                                             h_sb[:, kt * TOK:(kt + 1) * TOK],
                                             start=kt == 0, stop=kt == KT2 - 1)
                        nc.vector.tensor_add(x_sb[:, ft, :], x_sb[:, ft, :], ps[:])

                    if l + 1 < L:
                        w2_next = load_w2(l + 1)
                        wa_sb, w1_sb, w2_sb = wa_next, w1_next, w2_next

            # ---- final: allgather x, vocab-sharded projection ----
            with (
                tc.tile_pool(name="fin", bufs=1) as fin,
                tc.tile_pool(name="wop", bufs=2) as wop,
                tc.tile_pool(name="lgp", bufs=2) as lgp,
            ):
                xb_sb = fin.tile([128, KT, TOK], BF16)
                for kt in range(KT):
                    nc.vector.tensor_copy(xb_sb[:, kt, :], x_sb[:, kt, :])
                xf = dram.tile([128, KT * TOK], BF16, tag="xf")
                nc.sync.dma_start(xf[:], xb_sb[:].rearrange("p t n -> p (t n)"))
                xg = dram.tile([NC * 128, KT * TOK], BF16, tag="xg", addr_space="Shared")
                nc.gpsimd.collective_compute(
                    "AllGather", mybir.AluOpType.bypass, replica_groups=ALL,
                    ins=[xf.opt()], outs=[xg.opt()])
                xg_sb = fin.tile([128, NC, KT, TOK], BF16)
                nc.sync.dma_start(xg_sb[:], xg[:].rearrange(
                    "(r p) (t n) -> p r t n", p=128, t=KT))
                for c in range(4):
                    woc = wop.tile([128, KT, 1024], BF16, tag="wout")
                    nc.sync.dma_start(
                        woc[:], wout_in[c].rearrange("p (t f) -> p t f", t=KT))
                    lg = lgp.tile([128, 8, 1024], F32, tag="lgo")
                    for vt in range(8):
                        for hf in range(2):
                            ps = psW.tile([128, 512], F32, tag="lg")
                            for kt in range(KT):
                                nc.tensor.matmul(
                                    ps[:], woc[:, kt, vt * 128:(vt + 1) * 128],
                                    xg_sb[:, hf * 4:(hf + 1) * 4, kt, :],
                                    start=kt == 0, stop=kt == KT - 1)
                            nc.vector.tensor_copy(lg[:, vt, hf * 512:(hf + 1) * 512], ps[:])
                    nc.sync.dma_start(
                        out_d[c * 1024:(c + 1) * 1024, :]
                        .rearrange("(v p) n -> p v n", p=128), lg[:])
    nc.compile()
    _cached["nc"] = nc
    return nc


def _prep_inputs(inputs):
    tok = np.asarray(inputs["tokens"])
    x0 = np.asarray(inputs["tok_emb"], np.float32)[tok] + np.asarray(inputs["pos_emb"], np.float32)[None]
    x0 = x0.reshape(B * S, D)

    for name in ("bq", "bk", "bv", "bo", "b1", "b2", "b_out", "ln1_b", "ln2_b"):
        assert not np.any(np.asarray(inputs[name])), f"{name} expected to be all zeros"
    for name in ("ln1_s", "ln2_s"):
        assert np.all(np.asarray(inputs[name]) == 1.0), f"{name} expected to be all ones"

    f32 = lambda a: np.asarray(a, np.float32)
    wq = f32(inputs["Wq"]) / np.sqrt(DKH)
    wk, wv, wo = f32(inputs["Wk"]), f32(inputs["Wv"]), f32(inputs["Wo"])
    w1, w2 = f32(inputs["W1"]), f32(inputs["W2"])

    # attention weights: [L, 128, (t i f)] with value = W_i[l, t*128+p, f]
    wa = np.stack([wq, wk, wv, wo], axis=1)                       # [L, 4, D, D]
    wa = wa.reshape(L, 4, KT, 128, D).transpose(0, 3, 2, 1, 4)    # [L, p, t, i, f]
    wa = np.ascontiguousarray(wa.reshape(L, 128, KT * 4 * D)).astype(NPBF)
    w1p = w1.reshape(L, KT, 128, DFF).transpose(0, 2, 1, 3)
    w1p = np.ascontiguousarray(w1p.reshape(L, 128, KT * DFF)).astype(NPBF)
    w2p = w2.reshape(L, KT2, 128, D).transpose(0, 2, 1, 3)
    w2p = np.ascontiguousarray(w2p.reshape(L, 128, KT2 * D)).astype(NPBF)

    wout = f32(inputs["W_out"])                                   # [D, V]

    in_maps = []
    for c in range(NC):
        wc = np.zeros((D, VPAD), np.float32)
        wc[:, :VSH] = wout[:, c * VSH:(c + 1) * VSH]
        # [4, 128, (t f)] with value = wc[t*128+p, c4*1024+f]
        wop = wc.reshape(KT, 128, 4, 1024).transpose(2, 1, 0, 3)
        wop = np.ascontiguousarray(wop.reshape(4, 128, KT * 1024)).astype(NPBF)
        # x0 feature-major [p, (t n)] = x0[token n, t*128+p]
        xc = x0[c * TOK:(c + 1) * TOK].T.reshape(KT, 128, TOK).transpose(1, 0, 2)
        xc = np.ascontiguousarray(xc.reshape(128, KT * TOK))
        in_maps.append({"x0": xc, "wa": wa, "w1": w1p, "w2": w2p, "wout": wop})
    return in_maps


def _assemble(results):
    parts = [np.asarray(results[c]["logits"][:VSH]) for c in range(NC)]
    logits = np.concatenate(parts, axis=0)          # [V, B*S]
    return np.ascontiguousarray(logits.T).reshape(B, S, V).astype(np.float32)


def _run(inputs, **kw):
    nc = _build()
    in_maps = _prep_inputs(inputs)
    res = bass_utils.run_bass_kernel_spmd(nc, in_maps, core_ids=list(range(NC)), **kw)
    return _assemble(res.results), res


def kernel(**inputs):
    out, _ = _run(inputs)
    return out
